# revision 25
# baseline (speedup 1.0000x reference)
"""Trainium2 Bass kernel for nn_Attention_22179211117150 (sparse axial attention).

Strategy (8 NeuronCores, zero collectives):
  - Axial attention: tokens attend within their own frame (N=1024 tokens,
    F=16 frames). 2 frames per core; weights replicated; fully local.
  - Keys/values compressed on host to the kept (mask!=0) positions.
  - All matmuls bf16 (fp32 psum); softmax exp in f32 on ScalarE.
  - Transposed dataflow: qT/kT [d, tokens], simT [keys, queries].
  - Per (frame, head-pair, key-tile): two ping-pong [128, 1024] psum sim
    tiles (one per head of the pair), each drained by one [128, 1024] exp
    activation into the shared [128, 2048] ET tile, so the PE<->ACT chain
    double-buffers and semaphore latency hides.
  - Diagonal mask: narrow band multiply on GpSimd over a [128, 2, bw]
    strided view of ET covering both heads in one instruction (keeps the
    loaded VectorE free for psum drains).
  - Softmax denominators ride the av matmul as a ones-column (M=65);
    cast + K=1 broadcast matmul + reciprocal + fused normalize-cast.
  - Demand-driven emission: projection/out-proj/av work is queued as fill
    units and drained between sim key-tiles so the in-order PE queue always
    has work while ScalarE chews exps; ScalarE starts ~15us in.
"""
import numpy as np
import ml_dtypes
from collections import deque
from contextlib import ExitStack

import concourse.bass as bass
import concourse.mybir as mybir
import concourse.tile as tile
from concourse import bacc
from concourse.bass_utils import run_bass_kernel_spmd

dt = mybir.dt
AF = mybir.ActivationFunctionType
bf16 = ml_dtypes.bfloat16

B, F, N, H, D, DIM = 1, 16, 1024, 8, 64, 512
NCORES = 8
FPC = F // NCORES          # frames per core
T = FPC * N                # tokens per core
NEG = -1.0e9

BAND_ON_GPSIMD = True
BCAST_GPSIMD = False       # gpsimd partition_broadcast: NaN output + 3x slowdown, rejected
FP8_QK = False             # fp8 e4m3 DoubleRow q/k: passes (1.64e-2) but no wall-time gain; proj PE time was already hidden
FP8_S = 64.0               # weight pre-scale so fp8 values stay normal

TRACE = False
LAST = {}

_nc_cache = {}


def _build(njt, diag, band_lo, band_w):
    nkp = njt * 128
    KV = FPC * nkp                     # kv rows per core (padded)
    nc = bacc.Bacc("TRN2", target_bir_lowering=False, debug=False,
                   num_devices=NCORES)

    f8 = dt.float8e4
    if FP8_QK:
        # x / weights for q,k in fp8, contraction chunk-PAIRED for DoubleRow
        xT_d = nc.declare_dram_parameter("xT", [128, 4 * T], f8, isOutput=False)
        xkv8_d = nc.declare_dram_parameter("xkv8", [128, 4 * KV], f8, isOutput=False)
        wq_d = nc.declare_dram_parameter("wq", [128, 4 * 512], f8, isOutput=False)
        wk_d = nc.declare_dram_parameter("wk", [128, 4 * 512], f8, isOutput=False)
    else:
        xT_d = nc.declare_dram_parameter("xT", [128, 4 * T], dt.bfloat16, isOutput=False)
        wq_d = nc.declare_dram_parameter("wq", [128, 4 * 512], dt.bfloat16, isOutput=False)
        wk_d = nc.declare_dram_parameter("wk", [128, 4 * 512], dt.bfloat16, isOutput=False)
    xkvT_d = nc.declare_dram_parameter("xkvT", [128, 4 * KV], dt.bfloat16, isOutput=False)
    wv_d = nc.declare_dram_parameter("wv", [128, 4 * 512], dt.bfloat16, isOutput=False)
    wo_d = nc.declare_dram_parameter("wo", [128, 4 * 512], dt.bfloat16, isOutput=False)
    eb_d = nc.declare_dram_parameter("eb", [128, njt], dt.float32, isOutput=False)
    if diag:
        mmb_d = nc.declare_dram_parameter("mmb", [128, njt * 2 * band_w],
                                          dt.bfloat16, isOutput=False)
    out_d = nc.declare_dram_parameter("out", [T, DIM], dt.float32, isOutput=True)

    with tile.TileContext(nc) as tc, ExitStack() as ctx:
        consts = ctx.enter_context(tc.tile_pool(name="consts", bufs=1))
        work = ctx.enter_context(tc.tile_pool(name="work", bufs=1))
        etp = ctx.enter_context(tc.tile_pool(name="etp", bufs=12))
        smallp = ctx.enter_context(tc.tile_pool(name="small", bufs=6))
        outp = ctx.enter_context(tc.tile_pool(name="outp", bufs=3))
        dramp = ctx.enter_context(tc.tile_pool(name="dramp", bufs=2, space="DRAM"))
        simp = ctx.enter_context(tc.tile_pool(name="simp", bufs=2, space="PSUM"))
        avp = ctx.enter_context(tc.tile_pool(name="avp", bufs=2, space="PSUM"))
        pp = ctx.enter_context(tc.tile_pool(name="pp", bufs=2, space="PSUM"))

        def load(d, shape, dtype, tag, eng=None, frame_split=False):
            eng = eng or nc.sync
            t = consts.tile(shape, dtype, tag=tag, name=tag)
            n = shape[1]
            if frame_split:
                # 4 contraction chunks x FPC frame-halves; frame-0 halves
                # first so frame-0 projections start as early as possible.
                half = n // 8
                for fh in range(2):
                    for cc in range(4):
                        o = cc * (n // 4) + fh * half
                        eng.dma_start(t[:, o:o + half], d[:, o:o + half])
            else:
                eng.dma_start(t[:], d[:])
            return t

        qk_dt = dt.float8e4 if FP8_QK else dt.bfloat16
        # frame-0 data + weights split across the sync and scalar issue
        # queues; frame-1 pieces ride the otherwise-idle gpsimd queue.
        wq = load(wq_d, [128, 4 * 512], qk_dt, "wq")
        wk = load(wk_d, [128, 4 * 512], qk_dt, "wk", eng=nc.scalar)
        eb = load(eb_d, [128, njt], dt.float32, "eb", eng=nc.scalar)
        xT = consts.tile([128, 4 * T], qk_dt, tag="xT", name="xT")
        xkvT = consts.tile([128, 4 * KV], dt.bfloat16, tag="xkvT", name="xkvT")
        for cc in range(4):
            nc.sync.dma_start(xT[:, cc * T: cc * T + N],
                              xT_d[:, cc * T: cc * T + N])
        for cc in range(4):
            nc.scalar.dma_start(xkvT[:, cc * KV: cc * KV + nkp],
                                xkvT_d[:, cc * KV: cc * KV + nkp])
        wv = load(wv_d, [128, 4 * 512], dt.bfloat16, "wv", eng=nc.scalar)
        for cc in range(4):
            nc.gpsimd.dma_start(xT[:, cc * T + N: (cc + 1) * T],
                                xT_d[:, cc * T + N: (cc + 1) * T])
        for cc in range(4):
            nc.gpsimd.dma_start(xkvT[:, cc * KV + nkp: (cc + 1) * KV],
                                xkvT_d[:, cc * KV + nkp: (cc + 1) * KV])
        wo = load(wo_d, [128, 4 * 512], dt.bfloat16, "wo")
        if diag:
            mmb = load(mmb_d, [128, njt * 2 * band_w], dt.bfloat16, "mmb",
                       eng=nc.gpsimd)
        xk_src = xkvT

        ones_sb = work.tile([128, 64], dt.bfloat16, tag="ones", name="ones")
        nc.vector.memset(ones_sb[:], 1.0)

        # PE warm-up burst while inputs stream in (HAM clock gate).
        warm_src = work.tile([128, 512], dt.bfloat16, tag="warmsrc", name="warmsrc")
        nc.vector.memset(warm_src[:], 0.5)
        wps = pp.tile([128, 512], dt.float32, tag="pp", name="pp_t")
        for wi in range(20):
            nc.tensor.matmul(wps[0:64, :], ones_sb[:, 0:64], warm_src[:],
                             start=(wi == 0), stop=(wi == 19))
        wsb = smallp.tile([1, 64], dt.float32, tag="warm", name="warm_t")
        nc.vector.tensor_copy(wsb[:], wps[0:1, 0:64])
        wdr = dramp.tile([1, 64], dt.float32, tag="wdr", name="wdr_t")
        nc.sync.dma_start(wdr[:], wsb[:])

        qT = [work.tile([128, T], dt.bfloat16, tag=f"qT{hp}", name=f"qT{hp}")
              for hp in range(4)]
        kT = [work.tile([128, KV], dt.bfloat16, tag=f"kT{hp}", name=f"kT{hp}")
              for hp in range(4)]
        vt = [[work.tile([128, 8 * 65], dt.bfloat16, tag=f"v{f}_{jt}",
                         name=f"v{f}_{jt}") for jt in range(njt)]
              for f in range(FPC)]
        aoT = [work.tile([128, T], dt.bfloat16, tag=f"aoT{hp}", name=f"aoT{hp}")
               for hp in range(4)]

        kwins = [(0, 512)] + ([(512, nkp - 512)] if nkp > 512 else [])

        # ---- emitters ----
        DR = mybir.MatmulPerfMode.DoubleRow

        def emit_q_slice(f, hp, iw):
            w0 = f * N + iw * 512
            ps = pp.tile([128, 512], dt.float32, tag="pp", name="pp_t")
            if FP8_QK:
                for g in (0, 1):
                    w3 = wq[:, g * 1024 + hp * 256: g * 1024 + hp * 256 + 256
                            ].rearrange("p (j m) -> p j m", j=2)
                    x3 = xT[:, g * 2 * T: (g + 1) * 2 * T
                            ].rearrange("p (j t) -> p j t", j=2)
                    nc.tensor.matmul(ps[:], w3[:, :, :], x3[:, :, w0:w0 + 512],
                                     start=(g == 0), stop=(g == 1),
                                     perf_mode=DR)
                nc.vector.tensor_scalar_mul(qT[hp][:, w0:w0 + 512], ps[:],
                                            1.0 / FP8_S)
            else:
                for cc in range(4):
                    nc.tensor.matmul(
                        ps[:],
                        wq[:, cc * 512 + hp * 128: cc * 512 + hp * 128 + 128],
                        xT[:, cc * T + w0: cc * T + w0 + 512],
                        start=(cc == 0), stop=(cc == 3))
                nc.vector.tensor_copy(qT[hp][:, w0:w0 + 512], ps[:])

        def emit_k_slice(f, hp, win):
            w0, wl = win
            c0 = f * nkp + w0
            ps = pp.tile([128, 512], dt.float32, tag="pp", name="pp_t")
            if FP8_QK:
                for g in (0, 1):
                    w3 = wk[:, g * 1024 + hp * 256: g * 1024 + hp * 256 + 256
                            ].rearrange("p (j m) -> p j m", j=2)
                    x3 = xk_src[:, g * 2 * KV: (g + 1) * 2 * KV
                                ].rearrange("p (j t) -> p j t", j=2)
                    nc.tensor.matmul(ps[:, 0:wl], w3[:, :, :],
                                     x3[:, :, c0:c0 + wl],
                                     start=(g == 0), stop=(g == 1),
                                     perf_mode=DR)
                nc.vector.tensor_scalar_mul(kT[hp][:, c0:c0 + wl],
                                            ps[:, 0:wl], 1.0 / FP8_S)
            else:
                for cc in range(4):
                    nc.tensor.matmul(
                        ps[:, 0:wl],
                        wk[:, cc * 512 + hp * 128: cc * 512 + hp * 128 + 128],
                        xkvT[:, cc * KV + c0: cc * KV + c0 + wl],
                        start=(cc == 0), stop=(cc == 3))
                nc.vector.tensor_copy(kT[hp][:, c0:c0 + wl], ps[:, 0:wl])

        def emit_v_slice(f, jt):
            col0 = f * nkp + jt * 128
            ps = pp.tile([128, 512], dt.float32, tag="pp", name="pp_t")
            for cc in range(4):
                nc.tensor.matmul(ps[:],
                                 xkvT[:, cc * KV + col0: cc * KV + col0 + 128],
                                 wv[:, cc * 512: cc * 512 + 512],
                                 start=(cc == 0), stop=(cc == 3))
            v3 = vt[f][jt][:, :].rearrange("p (h c) -> p h c", c=65)
            p3 = ps[:, :].rearrange("p (h c) -> p h c", c=64)
            nc.vector.tensor_copy(v3[:, :, 0:64], p3[:, :, :])
            nc.vector.memset(v3[:, :, 64:65], 1.0)

        def emit_out_slice(f, tt):
            tg = f * (N // 128) + tt
            ps = pp.tile([128, 512], dt.float32, tag="pp", name="pp_t")
            for hp in range(4):
                nc.tensor.matmul(ps[:],
                                 aoT[hp][:, tg * 128:(tg + 1) * 128],
                                 wo[:, hp * 512:(hp + 1) * 512],
                                 start=(hp == 0), stop=(hp == 3))
            osb = outp.tile([128, 512], dt.float32, tag="osb", name="osb_t")
            nc.vector.tensor_copy(osb[:], ps[:])
            nc.sync.dma_start(out_d[tg * 128:(tg + 1) * 128, :], osb[:])

        def emit_av_combo(f, hp, ET, hr, iw):
            h = hp * 2 + hr
            ps = avp.tile([128, 512], dt.float32, tag="av", name="av_t")
            for jt in range(njt):
                nc.tensor.matmul(
                    ps[0:65, :],
                    vt[f][jt][:, 65 * h: 65 * h + 65],
                    ET[jt][:, hr * 1024 + iw * 512: hr * 1024 + iw * 512 + 512],
                    start=(jt == 0), stop=(jt == njt - 1))
            win = slice(f * N + iw * 512, f * N + iw * 512 + 512)
            if False:
                pass
            else:
                s_sb = smallp.tile([128, 512], dt.bfloat16, tag="sr", name="sr_t")
                nc.vector.tensor_copy(s_sb[64:65, :], ps[64:65, :])
                psx = pp.tile([128, 512], dt.float32, tag="pp", name="pp_t")
                nc.tensor.matmul(psx[0:64, :], ones_sb[64:65, 0:64],
                                 s_sb[64:65, :], start=True, stop=True)
                sr = smallp.tile([64, 512], dt.float32, tag="srec", name="srec_t")
                nc.vector.reciprocal_approx_fast(sr[:], psx[0:64, :])
            if hr == 0:
                nc.vector.tensor_mul(aoT[hp][0:64, win], ps[0:64, :], sr[0:64, :])
            else:
                sc = smallp.tile([64, 512], dt.bfloat16, tag="aosc", name="aosc_t")
                nc.vector.tensor_mul(sc[:], ps[0:64, :], sr[0:64, :])
                nc.sync.dma_start(aoT[hp][64:128, win], sc[:])

        def emit_sim_hr(f, hp, jt, hr, et):
            st = simp.tile([128, 1024], dt.float32, tag="sim", name="sim_t")
            k0 = f * nkp + jt * 128
            po = 64 * hr
            for iw in (0, 1):
                nc.tensor.matmul(
                    st[:, iw * 512: iw * 512 + 512],
                    kT[hp][po:po + 64, k0:k0 + 128],
                    qT[hp][po:po + 64, f * N + iw * 512: f * N + iw * 512 + 512],
                    start=True, stop=True)
            nc.scalar.activation(et[:, hr * 1024:(hr + 1) * 1024], st[:],
                                 AF.Exp, bias=eb[:, jt:jt + 1])

        def emit_band(jt, et):
            lo = band_lo[jt]
            e3 = et[:, :].rearrange("p (h q) -> p h q", h=2)
            m3 = mmb[:, jt * 2 * band_w:(jt + 1) * 2 * band_w].rearrange(
                "p (h w) -> p h w", h=2)
            eng = nc.gpsimd if BAND_ON_GPSIMD else nc.vector
            eng.tensor_mul(e3[:, :, lo:lo + band_w], e3[:, :, lo:lo + band_w],
                           m3[:, :, :])

        # ---- demand-driven schedule ----
        fills = deque()
        for hp in range(4):
            for iw in range(2):
                fills.append(('q', 0, hp, iw))
        for hp in range(4):
            for wi, win in enumerate(kwins):
                fills.append(('k', 0, hp, wi))
        for jt in range(njt):
            fills.append(('v', 0, jt))
        for hp in range(4):
            for iw in range(2):
                fills.append(('q', 1, hp, iw))
        for hp in range(4):
            for wi, win in enumerate(kwins):
                fills.append(('k', 1, hp, wi))
        for jt in range(njt):
            fills.append(('v', 1, jt))

        def run_fill(u):
            kind = u[0]
            if kind == 'q':
                emit_q_slice(u[1], u[2], u[3])
            elif kind == 'k':
                emit_k_slice(u[1], u[2], kwins[u[3]])
            elif kind == 'v':
                emit_v_slice(u[1], u[2])
            elif kind == 'o':
                emit_out_slice(u[1], u[2])

        def drain_matching(pred):
            rest = deque()
            while fills:
                u = fills.popleft()
                if pred(u):
                    run_fill(u)
                else:
                    rest.append(u)
            fills.extend(rest)

        def drain_some(k):
            for _ in range(k):
                if fills:
                    run_fill(fills.popleft())

        prev = None
        groups = [(f, hp) for f in range(FPC) for hp in range(4)]
        for g, (f, hp) in enumerate(groups):
            # prerequisites: q/k slices of this group's (f, hp)
            drain_matching(lambda u: u[0] in ('q', 'k') and u[1] == f
                           and u[2] == hp)
            if prev is not None:
                # av of prev group needs its frame's v tiles
                drain_matching(lambda u: u[0] == 'v' and u[1] == prev[0])
            ET = {jt: etp.tile([128, 2048], dt.bfloat16, tag="et", name="et_t")
                  for jt in range(njt)}
            for jt in range(njt):
                emit_sim_hr(f, hp, jt, 0, ET[jt])
                if prev is not None and jt < 4:
                    pf, php, pET = prev
                    emit_av_combo(pf, php, pET, jt // 2, jt % 2)
                else:
                    drain_some(2)
                emit_sim_hr(f, hp, jt, 1, ET[jt])
                if diag:
                    emit_band(jt, ET[jt])
                drain_some(1)
            if prev is not None and prev[1] == 3:
                for tt in range(N // 128):
                    fills.append(('o', prev[0], tt))
            prev = (f, hp, ET)

        # final drain: iw0 combos first so the first half-frame's
        # out-projection overlaps the iw1 combos.
        pf, php, pET = prev
        emit_av_combo(pf, php, pET, 0, 0)
        emit_av_combo(pf, php, pET, 1, 0)
        while fills:
            run_fill(fills.popleft())
        emit_av_combo(pf, php, pET, 0, 1)
        for tt in range(4):
            emit_out_slice(pf, tt)
        emit_av_combo(pf, php, pET, 1, 1)
        for tt in range(4, N // 128):
            emit_out_slice(pf, tt)

    nc.compile()
    return nc


def _chunk_major(a):
    """[512, M] f32 -> [128, 4*M] bf16, contraction chunk-major."""
    m = a.shape[1]
    return np.ascontiguousarray(
        a.reshape(4, 128, m).transpose(1, 0, 2).reshape(128, 4 * m)).astype(bf16)


def kernel(x, W_qkv, W_out, mask, diag):
    x = np.asarray(x, dtype=np.float32).reshape(F * N, DIM)
    W_qkv = np.asarray(W_qkv, dtype=np.float32)
    W_out = np.asarray(W_out, dtype=np.float32)
    maskv = np.asarray(mask).reshape(N)
    diag = int(np.asarray(diag))

    kept = np.flatnonzero(maskv != 0)
    nk = int(kept.size)
    assert nk > 0, "all-masked input not supported"
    njt = (nk + 127) // 128
    nkp = njt * 128

    Wq = W_qkv[:, 0:512] * np.float32(D ** -0.5)
    Wk = W_qkv[:, 512:1024]
    Wv = W_qkv[:, 1024:1536]

    if FP8_QK:
        f8 = ml_dtypes.float8_e4m3fn
        S = np.float32(FP8_S)

        def _dr_weights(W_eff):
            # DoubleRow layout: per (g, hp) a [128, 2, 128] block with the
            # contraction chunk PAIR (2g, 2g+1) interleaved along free dim.
            cm = np.ascontiguousarray(W_eff.reshape(4, 128, 512))
            out = np.zeros((128, 4 * 512), np.float32)
            for g in range(2):
                for hp in range(4):
                    base = g * 1024 + hp * 256
                    for j in range(2):
                        out[:, base + j * 128: base + (j + 1) * 128] =                             cm[2 * g + j][:, hp * 128:(hp + 1) * 128]
            return (out * S).astype(f8)

        wq_h = _dr_weights(Wq)
        wk_h = _dr_weights(Wk)
    else:
        wq_h = _chunk_major(Wq)
        wk_h = _chunk_major(Wk)
    wv_h = _chunk_major(Wv)
    wo_h = _chunk_major(W_out)

    eb_h = np.zeros((128, njt), np.float32)
    for jt in range(njt):
        rows = np.arange(jt * 128, jt * 128 + 128)
        eb_h[:, jt] = np.where(rows < nk, 0.0, NEG)

    if diag:
        los, ws = [], []
        for jt in range(njt):
            idx = kept[jt * 128: min(jt * 128 + 128, nk)]
            lo = int(idx.min()) & ~1
            los.append(lo)
            ws.append(int(idx.max()) + 1 - lo)
        bw = (max(ws) + 1) & ~1
        los = [min(lo, N - bw) for lo in los]
        mmb_h = np.ones((128, njt * 2 * bw), np.float32)
        for jt in range(njt):
            valid = min(128, nk - jt * 128)
            p = np.arange(valid)
            off = kept[jt * 128: jt * 128 + valid] - los[jt]
            mmb_h[p, jt * 2 * bw + off] = 0.0
            mmb_h[p, jt * 2 * bw + bw + off] = 0.0
        mmb_h = mmb_h.astype(bf16)
        band_lo = tuple(los)
    else:
        bw = 0
        band_lo = None
        mmb_h = None

    key = (njt, diag, bw, band_lo)
    if key not in _nc_cache:
        _nc_cache[key] = _build(njt, diag, band_lo, bw)
    nc = _nc_cache[key]

    xbf = x.astype(bf16)
    in_maps = []
    for m in range(NCORES):
        xs = xbf[m * T:(m + 1) * T]                      # [T, DIM] bf16
        xsT32 = np.ascontiguousarray(xs.T.astype(np.float32))
        kvrows = np.zeros((FPC * nkp, DIM), np.float32)
        for f in range(FPC):
            kvrows[f * nkp: f * nkp + nk] = xs[f * N + kept].astype(np.float32)
        kvT32 = np.ascontiguousarray(kvrows.T)
        if FP8_QK:
            f8 = ml_dtypes.float8_e4m3fn
            xm = x[m * T:(m + 1) * T]                    # fp32 source
            xT_h = _chunk_major(np.ascontiguousarray(xm.T)).astype(np.float32)
            xT_h = np.ascontiguousarray(xm.T).reshape(4, 128, T).transpose(
                1, 0, 2).reshape(128, 4 * T).astype(f8)
            kv32 = np.zeros((FPC * nkp, DIM), np.float32)
            for f in range(FPC):
                kv32[f * nkp: f * nkp + nk] = xm[f * N + kept]
            xkv8_h = np.ascontiguousarray(kv32.T).reshape(4, 128, FPC * nkp
                ).transpose(1, 0, 2).reshape(128, 4 * FPC * nkp).astype(f8)
        else:
            xT_h = _chunk_major(xsT32)
        xkvT_h = _chunk_major(kvT32)
        im = dict(xT=xT_h, xkvT=xkvT_h, wq=wq_h, wk=wk_h, wv=wv_h, wo=wo_h,
                  eb=eb_h)
        if FP8_QK:
            im["xkv8"] = xkv8_h
        if diag:
            im["mmb"] = mmb_h
        in_maps.append(im)

    core_ids = list(range(NCORES))
    if TRACE:
        r = run_bass_kernel_spmd(nc, in_maps, core_ids, trace=True)
        LAST["exec_time_ns"] = r.exec_time_ns
        LAST["results"] = r
        results = r.results
    else:
        results = None
        for attempt in range(3):
            try:
                results = run_bass_kernel_spmd(nc, in_maps, core_ids).results
                break
            except Exception:
                if attempt == 2:
                    raise
                import time as _time
                _time.sleep(2.0)

    out = np.concatenate([np.asarray(results[m]["out"]) for m in range(NCORES)],
                         axis=0)
    return out.reshape(B, F * N, DIM).astype(np.float32)


# revision 26
# speedup vs baseline: 1.0003x; 1.0003x over previous
"""Trainium2 Bass kernel for nn_Attention_22179211117150 (sparse axial attention).

Strategy (8 NeuronCores, zero collectives):
  - Axial attention: tokens attend within their own frame (N=1024 tokens,
    F=16 frames). 2 frames per core; weights replicated; fully local.
  - Keys/values compressed on host to the kept (mask!=0) positions.
  - All matmuls bf16 (fp32 psum); softmax exp in f32 on ScalarE.
  - Transposed dataflow: qT/kT [d, tokens], simT [keys, queries].
  - Per (frame, head-pair, key-tile): two ping-pong [128, 1024] psum sim
    tiles (one per head of the pair), each drained by one [128, 1024] exp
    activation into the shared [128, 2048] ET tile, so the PE<->ACT chain
    double-buffers and semaphore latency hides.
  - Diagonal mask: narrow band multiply on GpSimd over a [128, 2, bw]
    strided view of ET covering both heads in one instruction (keeps the
    loaded VectorE free for psum drains).
  - Softmax denominators ride the av matmul as a ones-column (M=65);
    cast + K=1 broadcast matmul + reciprocal + fused normalize-cast.
  - Demand-driven emission: projection/out-proj/av work is queued as fill
    units and drained between sim key-tiles so the in-order PE queue always
    has work while ScalarE chews exps; ScalarE starts ~15us in.
"""
import numpy as np
import ml_dtypes
from collections import deque
from contextlib import ExitStack

import concourse.bass as bass
import concourse.mybir as mybir
import concourse.tile as tile
from concourse import bacc
from concourse.bass_utils import run_bass_kernel_spmd

dt = mybir.dt
AF = mybir.ActivationFunctionType
bf16 = ml_dtypes.bfloat16

B, F, N, H, D, DIM = 1, 16, 1024, 8, 64, 512
NCORES = 8
FPC = F // NCORES          # frames per core
T = FPC * N                # tokens per core
NEG = -1.0e9

BAND_ON_GPSIMD = True
BCAST_GPSIMD = False       # gpsimd partition_broadcast: NaN output + 3x slowdown, rejected
FP8_QK = False             # fp8 e4m3 DoubleRow q/k: passes (1.64e-2) but no wall-time gain; proj PE time was already hidden
FP8_S = 64.0               # weight pre-scale so fp8 values stay normal

TRACE = False
LAST = {}

_nc_cache = {}


def _build(njt, diag, band_lo, band_w):
    nkp = njt * 128
    KV = FPC * nkp                     # kv rows per core (padded)
    nc = bacc.Bacc("TRN2", target_bir_lowering=False, debug=False,
                   num_devices=NCORES)

    f8 = dt.float8e4
    if FP8_QK:
        # x / weights for q,k in fp8, contraction chunk-PAIRED for DoubleRow
        xT_d = nc.declare_dram_parameter("xT", [128, 4 * T], f8, isOutput=False)
        xkv8_d = nc.declare_dram_parameter("xkv8", [128, 4 * KV], f8, isOutput=False)
        wq_d = nc.declare_dram_parameter("wq", [128, 4 * 512], f8, isOutput=False)
        wk_d = nc.declare_dram_parameter("wk", [128, 4 * 512], f8, isOutput=False)
    else:
        xT_d = nc.declare_dram_parameter("xT", [128, 4 * T], dt.bfloat16, isOutput=False)
        wq_d = nc.declare_dram_parameter("wq", [128, 4 * 512], dt.bfloat16, isOutput=False)
        wk_d = nc.declare_dram_parameter("wk", [128, 4 * 512], dt.bfloat16, isOutput=False)
    xkvT_d = nc.declare_dram_parameter("xkvT", [128, 4 * KV], dt.bfloat16, isOutput=False)
    wv_d = nc.declare_dram_parameter("wv", [128, 4 * 512], dt.bfloat16, isOutput=False)
    wo_d = nc.declare_dram_parameter("wo", [128, 4 * 512], dt.bfloat16, isOutput=False)
    eb_d = nc.declare_dram_parameter("eb", [128, njt], dt.float32, isOutput=False)
    if diag:
        mmb_d = nc.declare_dram_parameter("mmb", [128, njt * 2 * band_w],
                                          dt.bfloat16, isOutput=False)
    out_d = nc.declare_dram_parameter("out", [T, DIM], dt.float32, isOutput=True)

    with tile.TileContext(nc) as tc, ExitStack() as ctx:
        consts = ctx.enter_context(tc.tile_pool(name="consts", bufs=1))
        work = ctx.enter_context(tc.tile_pool(name="work", bufs=1))
        etp = ctx.enter_context(tc.tile_pool(name="etp", bufs=12))
        smallp = ctx.enter_context(tc.tile_pool(name="small", bufs=6))
        outp = ctx.enter_context(tc.tile_pool(name="outp", bufs=3))
        dramp = ctx.enter_context(tc.tile_pool(name="dramp", bufs=2, space="DRAM"))
        simp = ctx.enter_context(tc.tile_pool(name="simp", bufs=2, space="PSUM"))
        avp = ctx.enter_context(tc.tile_pool(name="avp", bufs=2, space="PSUM"))
        pp = ctx.enter_context(tc.tile_pool(name="pp", bufs=2, space="PSUM"))

        def load(d, shape, dtype, tag, eng=None, frame_split=False):
            eng = eng or nc.sync
            t = consts.tile(shape, dtype, tag=tag, name=tag)
            n = shape[1]
            if frame_split:
                # 4 contraction chunks x FPC frame-halves; frame-0 halves
                # first so frame-0 projections start as early as possible.
                half = n // 8
                for fh in range(2):
                    for cc in range(4):
                        o = cc * (n // 4) + fh * half
                        eng.dma_start(t[:, o:o + half], d[:, o:o + half])
            else:
                eng.dma_start(t[:], d[:])
            return t

        qk_dt = dt.float8e4 if FP8_QK else dt.bfloat16
        # frame-0 data + weights split across the sync and scalar issue
        # queues; frame-1 pieces ride the otherwise-idle gpsimd queue.
        wq = load(wq_d, [128, 4 * 512], qk_dt, "wq")
        wk = load(wk_d, [128, 4 * 512], qk_dt, "wk", eng=nc.scalar)
        eb = load(eb_d, [128, njt], dt.float32, "eb", eng=nc.scalar)
        xT = consts.tile([128, 4 * T], qk_dt, tag="xT", name="xT")
        xkvT = consts.tile([128, 4 * KV], dt.bfloat16, tag="xkvT", name="xkvT")
        for cc in range(4):
            nc.sync.dma_start(xT[:, cc * T: cc * T + N],
                              xT_d[:, cc * T: cc * T + N])
        for cc in range(4):
            nc.scalar.dma_start(xkvT[:, cc * KV: cc * KV + nkp],
                                xkvT_d[:, cc * KV: cc * KV + nkp])
        wv = load(wv_d, [128, 4 * 512], dt.bfloat16, "wv", eng=nc.scalar)
        for cc in range(4):
            nc.gpsimd.dma_start(xT[:, cc * T + N: (cc + 1) * T],
                                xT_d[:, cc * T + N: (cc + 1) * T])
        for cc in range(4):
            nc.gpsimd.dma_start(xkvT[:, cc * KV + nkp: (cc + 1) * KV],
                                xkvT_d[:, cc * KV + nkp: (cc + 1) * KV])
        wo = load(wo_d, [128, 4 * 512], dt.bfloat16, "wo")
        if diag:
            mmb = load(mmb_d, [128, njt * 2 * band_w], dt.bfloat16, "mmb",
                       eng=nc.gpsimd)
        xk_src = xkvT

        ones_sb = work.tile([128, 64], dt.bfloat16, tag="ones", name="ones")
        nc.vector.memset(ones_sb[:], 1.0)

        # PE warm-up burst while inputs stream in (HAM clock gate).
        warm_src = work.tile([128, 512], dt.bfloat16, tag="warmsrc", name="warmsrc")
        nc.vector.memset(warm_src[:], 0.5)
        def emit_warm_burst(n):
            wps = pp.tile([128, 512], dt.float32, tag="pp", name="pp_t")
            for wi in range(n):
                nc.tensor.matmul(wps[0:64, :], ones_sb[:, 0:64], warm_src[:],
                                 start=(wi == 0), stop=(wi == n - 1))
            wsb = smallp.tile([1, 64], dt.float32, tag="warm", name="warm_t")
            nc.vector.tensor_copy(wsb[:], wps[0:1, 0:64])
            wdr = dramp.tile([1, 64], dt.float32, tag="wdr", name="wdr_t")
            nc.sync.dma_start(wdr[:], wsb[:])

        emit_warm_burst(10)

        qT = [work.tile([128, T], dt.bfloat16, tag=f"qT{hp}", name=f"qT{hp}")
              for hp in range(4)]
        kT = [work.tile([128, KV], dt.bfloat16, tag=f"kT{hp}", name=f"kT{hp}")
              for hp in range(4)]
        vt = [[work.tile([128, 8 * 65], dt.bfloat16, tag=f"v{f}_{jt}",
                         name=f"v{f}_{jt}") for jt in range(njt)]
              for f in range(FPC)]
        aoT = [work.tile([128, T], dt.bfloat16, tag=f"aoT{hp}", name=f"aoT{hp}")
               for hp in range(4)]

        kwins = [(0, 512)] + ([(512, nkp - 512)] if nkp > 512 else [])

        # ---- emitters ----
        DR = mybir.MatmulPerfMode.DoubleRow

        def emit_q_slice(f, hp, iw):
            w0 = f * N + iw * 512
            ps = pp.tile([128, 512], dt.float32, tag="pp", name="pp_t")
            if FP8_QK:
                for g in (0, 1):
                    w3 = wq[:, g * 1024 + hp * 256: g * 1024 + hp * 256 + 256
                            ].rearrange("p (j m) -> p j m", j=2)
                    x3 = xT[:, g * 2 * T: (g + 1) * 2 * T
                            ].rearrange("p (j t) -> p j t", j=2)
                    nc.tensor.matmul(ps[:], w3[:, :, :], x3[:, :, w0:w0 + 512],
                                     start=(g == 0), stop=(g == 1),
                                     perf_mode=DR)
                nc.vector.tensor_scalar_mul(qT[hp][:, w0:w0 + 512], ps[:],
                                            1.0 / FP8_S)
            else:
                for cc in range(4):
                    nc.tensor.matmul(
                        ps[:],
                        wq[:, cc * 512 + hp * 128: cc * 512 + hp * 128 + 128],
                        xT[:, cc * T + w0: cc * T + w0 + 512],
                        start=(cc == 0), stop=(cc == 3))
                nc.vector.tensor_copy(qT[hp][:, w0:w0 + 512], ps[:])

        def emit_k_slice(f, hp, win):
            w0, wl = win
            c0 = f * nkp + w0
            ps = pp.tile([128, 512], dt.float32, tag="pp", name="pp_t")
            if FP8_QK:
                for g in (0, 1):
                    w3 = wk[:, g * 1024 + hp * 256: g * 1024 + hp * 256 + 256
                            ].rearrange("p (j m) -> p j m", j=2)
                    x3 = xk_src[:, g * 2 * KV: (g + 1) * 2 * KV
                                ].rearrange("p (j t) -> p j t", j=2)
                    nc.tensor.matmul(ps[:, 0:wl], w3[:, :, :],
                                     x3[:, :, c0:c0 + wl],
                                     start=(g == 0), stop=(g == 1),
                                     perf_mode=DR)
                nc.vector.tensor_scalar_mul(kT[hp][:, c0:c0 + wl],
                                            ps[:, 0:wl], 1.0 / FP8_S)
            else:
                for cc in range(4):
                    nc.tensor.matmul(
                        ps[:, 0:wl],
                        wk[:, cc * 512 + hp * 128: cc * 512 + hp * 128 + 128],
                        xkvT[:, cc * KV + c0: cc * KV + c0 + wl],
                        start=(cc == 0), stop=(cc == 3))
                nc.vector.tensor_copy(kT[hp][:, c0:c0 + wl], ps[:, 0:wl])

        def emit_v_slice(f, jt):
            col0 = f * nkp + jt * 128
            ps = pp.tile([128, 512], dt.float32, tag="pp", name="pp_t")
            for cc in range(4):
                nc.tensor.matmul(ps[:],
                                 xkvT[:, cc * KV + col0: cc * KV + col0 + 128],
                                 wv[:, cc * 512: cc * 512 + 512],
                                 start=(cc == 0), stop=(cc == 3))
            v3 = vt[f][jt][:, :].rearrange("p (h c) -> p h c", c=65)
            p3 = ps[:, :].rearrange("p (h c) -> p h c", c=64)
            nc.vector.tensor_copy(v3[:, :, 0:64], p3[:, :, :])
            nc.vector.memset(v3[:, :, 64:65], 1.0)

        def emit_out_slice(f, tt):
            tg = f * (N // 128) + tt
            ps = pp.tile([128, 512], dt.float32, tag="pp", name="pp_t")
            for hp in range(4):
                nc.tensor.matmul(ps[:],
                                 aoT[hp][:, tg * 128:(tg + 1) * 128],
                                 wo[:, hp * 512:(hp + 1) * 512],
                                 start=(hp == 0), stop=(hp == 3))
            osb = outp.tile([128, 512], dt.float32, tag="osb", name="osb_t")
            nc.vector.tensor_copy(osb[:], ps[:])
            nc.sync.dma_start(out_d[tg * 128:(tg + 1) * 128, :], osb[:])

        def emit_av_combo(f, hp, ET, hr, iw):
            h = hp * 2 + hr
            ps = avp.tile([128, 512], dt.float32, tag="av", name="av_t")
            for jt in range(njt):
                nc.tensor.matmul(
                    ps[0:65, :],
                    vt[f][jt][:, 65 * h: 65 * h + 65],
                    ET[jt][:, hr * 1024 + iw * 512: hr * 1024 + iw * 512 + 512],
                    start=(jt == 0), stop=(jt == njt - 1))
            win = slice(f * N + iw * 512, f * N + iw * 512 + 512)
            if False:
                pass
            else:
                s_sb = smallp.tile([128, 512], dt.bfloat16, tag="sr", name="sr_t")
                nc.vector.tensor_copy(s_sb[64:65, :], ps[64:65, :])
                psx = pp.tile([128, 512], dt.float32, tag="pp", name="pp_t")
                nc.tensor.matmul(psx[0:64, :], ones_sb[64:65, 0:64],
                                 s_sb[64:65, :], start=True, stop=True)
                sr = smallp.tile([64, 512], dt.float32, tag="srec", name="srec_t")
                nc.vector.reciprocal_approx_fast(sr[:], psx[0:64, :])
            if hr == 0:
                nc.vector.tensor_mul(aoT[hp][0:64, win], ps[0:64, :], sr[0:64, :])
            else:
                sc = smallp.tile([64, 512], dt.bfloat16, tag="aosc", name="aosc_t")
                nc.vector.tensor_mul(sc[:], ps[0:64, :], sr[0:64, :])
                nc.sync.dma_start(aoT[hp][64:128, win], sc[:])

        def emit_sim_hr(f, hp, jt, hr, et):
            st = simp.tile([128, 1024], dt.float32, tag="sim", name="sim_t")
            k0 = f * nkp + jt * 128
            po = 64 * hr
            for iw in (0, 1):
                nc.tensor.matmul(
                    st[:, iw * 512: iw * 512 + 512],
                    kT[hp][po:po + 64, k0:k0 + 128],
                    qT[hp][po:po + 64, f * N + iw * 512: f * N + iw * 512 + 512],
                    start=True, stop=True)
            nc.scalar.activation(et[:, hr * 1024:(hr + 1) * 1024], st[:],
                                 AF.Exp, bias=eb[:, jt:jt + 1])

        def emit_band(jt, et):
            lo = band_lo[jt]
            e3 = et[:, :].rearrange("p (h q) -> p h q", h=2)
            m3 = mmb[:, jt * 2 * band_w:(jt + 1) * 2 * band_w].rearrange(
                "p (h w) -> p h w", h=2)
            eng = nc.gpsimd if BAND_ON_GPSIMD else nc.vector
            eng.tensor_mul(e3[:, :, lo:lo + band_w], e3[:, :, lo:lo + band_w],
                           m3[:, :, :])

        # ---- demand-driven schedule ----
        fills = deque()
        for hp in range(4):
            for iw in range(2):
                fills.append(('q', 0, hp, iw))
        for hp in range(4):
            for wi, win in enumerate(kwins):
                fills.append(('k', 0, hp, wi))
        for jt in range(njt):
            fills.append(('v', 0, jt))
        for hp in range(4):
            for iw in range(2):
                fills.append(('q', 1, hp, iw))
        for hp in range(4):
            for wi, win in enumerate(kwins):
                fills.append(('k', 1, hp, wi))
        for jt in range(njt):
            fills.append(('v', 1, jt))

        def run_fill(u):
            kind = u[0]
            if kind == 'q':
                emit_q_slice(u[1], u[2], u[3])
            elif kind == 'k':
                emit_k_slice(u[1], u[2], kwins[u[3]])
            elif kind == 'v':
                emit_v_slice(u[1], u[2])
            elif kind == 'o':
                emit_out_slice(u[1], u[2])
            elif kind == 'w':
                emit_warm_burst(u[1])

        def drain_matching(pred):
            rest = deque()
            while fills:
                u = fills.popleft()
                if pred(u):
                    run_fill(u)
                else:
                    rest.append(u)
            fills.extend(rest)

        def drain_some(k):
            for _ in range(k):
                if fills:
                    run_fill(fills.popleft())

        # fill the input-DMA wait window: small warm bursts between the
        # first projection slices keep the PE queue busy and HAM at 8/8
        fills.insert(2, ('w', 6))
        fills.insert(5, ('w', 6))
        fills.insert(8, ('w', 6))

        prev = None
        groups = [(f, hp) for f in range(FPC) for hp in range(4)]
        for g, (f, hp) in enumerate(groups):
            # prerequisites: q/k slices of this group's (f, hp)
            drain_matching(lambda u: u[0] in ('q', 'k') and u[1] == f
                           and u[2] == hp)
            if prev is not None:
                # av of prev group needs its frame's v tiles
                drain_matching(lambda u: u[0] == 'v' and u[1] == prev[0])
            ET = {jt: etp.tile([128, 2048], dt.bfloat16, tag="et", name="et_t")
                  for jt in range(njt)}
            for jt in range(njt):
                emit_sim_hr(f, hp, jt, 0, ET[jt])
                if prev is not None and jt < 4:
                    pf, php, pET = prev
                    emit_av_combo(pf, php, pET, jt // 2, jt % 2)
                else:
                    drain_some(2)
                emit_sim_hr(f, hp, jt, 1, ET[jt])
                if diag:
                    emit_band(jt, ET[jt])
                drain_some(1)
            if prev is not None and prev[1] == 3:
                for tt in range(N // 128):
                    fills.append(('o', prev[0], tt))
            prev = (f, hp, ET)

        # final drain: iw0 combos first so the first half-frame's
        # out-projection overlaps the iw1 combos.
        pf, php, pET = prev
        emit_av_combo(pf, php, pET, 0, 0)
        emit_av_combo(pf, php, pET, 1, 0)
        while fills:
            run_fill(fills.popleft())
        emit_av_combo(pf, php, pET, 0, 1)
        for tt in range(4):
            emit_out_slice(pf, tt)
        emit_av_combo(pf, php, pET, 1, 1)
        for tt in range(4, N // 128):
            emit_out_slice(pf, tt)

    nc.compile()
    return nc


def _chunk_major(a):
    """[512, M] f32 -> [128, 4*M] bf16, contraction chunk-major."""
    m = a.shape[1]
    return np.ascontiguousarray(
        a.reshape(4, 128, m).transpose(1, 0, 2).reshape(128, 4 * m)).astype(bf16)


def kernel(x, W_qkv, W_out, mask, diag):
    x = np.asarray(x, dtype=np.float32).reshape(F * N, DIM)
    W_qkv = np.asarray(W_qkv, dtype=np.float32)
    W_out = np.asarray(W_out, dtype=np.float32)
    maskv = np.asarray(mask).reshape(N)
    diag = int(np.asarray(diag))

    kept = np.flatnonzero(maskv != 0)
    nk = int(kept.size)
    assert nk > 0, "all-masked input not supported"
    njt = (nk + 127) // 128
    nkp = njt * 128

    Wq = W_qkv[:, 0:512] * np.float32(D ** -0.5)
    Wk = W_qkv[:, 512:1024]
    Wv = W_qkv[:, 1024:1536]

    if FP8_QK:
        f8 = ml_dtypes.float8_e4m3fn
        S = np.float32(FP8_S)

        def _dr_weights(W_eff):
            # DoubleRow layout: per (g, hp) a [128, 2, 128] block with the
            # contraction chunk PAIR (2g, 2g+1) interleaved along free dim.
            cm = np.ascontiguousarray(W_eff.reshape(4, 128, 512))
            out = np.zeros((128, 4 * 512), np.float32)
            for g in range(2):
                for hp in range(4):
                    base = g * 1024 + hp * 256
                    for j in range(2):
                        out[:, base + j * 128: base + (j + 1) * 128] =                             cm[2 * g + j][:, hp * 128:(hp + 1) * 128]
            return (out * S).astype(f8)

        wq_h = _dr_weights(Wq)
        wk_h = _dr_weights(Wk)
    else:
        wq_h = _chunk_major(Wq)
        wk_h = _chunk_major(Wk)
    wv_h = _chunk_major(Wv)
    wo_h = _chunk_major(W_out)

    eb_h = np.zeros((128, njt), np.float32)
    for jt in range(njt):
        rows = np.arange(jt * 128, jt * 128 + 128)
        eb_h[:, jt] = np.where(rows < nk, 0.0, NEG)

    if diag:
        los, ws = [], []
        for jt in range(njt):
            idx = kept[jt * 128: min(jt * 128 + 128, nk)]
            lo = int(idx.min()) & ~1
            los.append(lo)
            ws.append(int(idx.max()) + 1 - lo)
        bw = (max(ws) + 1) & ~1
        los = [min(lo, N - bw) for lo in los]
        mmb_h = np.ones((128, njt * 2 * bw), np.float32)
        for jt in range(njt):
            valid = min(128, nk - jt * 128)
            p = np.arange(valid)
            off = kept[jt * 128: jt * 128 + valid] - los[jt]
            mmb_h[p, jt * 2 * bw + off] = 0.0
            mmb_h[p, jt * 2 * bw + bw + off] = 0.0
        mmb_h = mmb_h.astype(bf16)
        band_lo = tuple(los)
    else:
        bw = 0
        band_lo = None
        mmb_h = None

    key = (njt, diag, bw, band_lo)
    if key not in _nc_cache:
        _nc_cache[key] = _build(njt, diag, band_lo, bw)
    nc = _nc_cache[key]

    xbf = x.astype(bf16)
    in_maps = []
    for m in range(NCORES):
        xs = xbf[m * T:(m + 1) * T]                      # [T, DIM] bf16
        xsT32 = np.ascontiguousarray(xs.T.astype(np.float32))
        kvrows = np.zeros((FPC * nkp, DIM), np.float32)
        for f in range(FPC):
            kvrows[f * nkp: f * nkp + nk] = xs[f * N + kept].astype(np.float32)
        kvT32 = np.ascontiguousarray(kvrows.T)
        if FP8_QK:
            f8 = ml_dtypes.float8_e4m3fn
            xm = x[m * T:(m + 1) * T]                    # fp32 source
            xT_h = _chunk_major(np.ascontiguousarray(xm.T)).astype(np.float32)
            xT_h = np.ascontiguousarray(xm.T).reshape(4, 128, T).transpose(
                1, 0, 2).reshape(128, 4 * T).astype(f8)
            kv32 = np.zeros((FPC * nkp, DIM), np.float32)
            for f in range(FPC):
                kv32[f * nkp: f * nkp + nk] = xm[f * N + kept]
            xkv8_h = np.ascontiguousarray(kv32.T).reshape(4, 128, FPC * nkp
                ).transpose(1, 0, 2).reshape(128, 4 * FPC * nkp).astype(f8)
        else:
            xT_h = _chunk_major(xsT32)
        xkvT_h = _chunk_major(kvT32)
        im = dict(xT=xT_h, xkvT=xkvT_h, wq=wq_h, wk=wk_h, wv=wv_h, wo=wo_h,
                  eb=eb_h)
        if FP8_QK:
            im["xkv8"] = xkv8_h
        if diag:
            im["mmb"] = mmb_h
        in_maps.append(im)

    core_ids = list(range(NCORES))
    if TRACE:
        r = run_bass_kernel_spmd(nc, in_maps, core_ids, trace=True)
        LAST["exec_time_ns"] = r.exec_time_ns
        LAST["results"] = r
        results = r.results
    else:
        results = None
        for attempt in range(3):
            try:
                results = run_bass_kernel_spmd(nc, in_maps, core_ids).results
                break
            except Exception:
                if attempt == 2:
                    raise
                import time as _time
                _time.sleep(2.0)

    out = np.concatenate([np.asarray(results[m]["out"]) for m in range(NCORES)],
                         axis=0)
    return out.reshape(B, F * N, DIM).astype(np.float32)
